# revision 25
# baseline (speedup 1.0000x reference)
"""NsNet2 single-step (fc1 + 2x GRU cell + 3x FC) Trainium2 kernel, v2.

Strategy (v2 — all-fp8 DoubleRow):
  - Pure data parallel: B=32768 -> 4096 rows/core on 8 cores; feature-major
    on chip ([feat, batch]) so matmul operands need no on-chip transposes.
  - EVERY matmul is fp8e4m3 + DoubleRow (0.5 cyc/row) with even K-chunk
    counts (K pads to 2-chunk multiples); numpy-sim predicts rel_err 1.6e-2.
  - Gate layout: zr weights have M=1024 = [z(400)|pad|r(400)|pad] so both
    gates land lane-aligned -> no r-realign DMAs. r-pairs are computed first
    so the n-gate chain can start early.
  - GRU2 contraction operand is one aligned block zr2op = [g1(512)|h2(512)]:
    chunks 0..3 written by GRU1's blend (fp8), chunks 4..7 DMA'd from padded
    h2; nh2/nx2 are chunk-slices of it (no separate h2 fp8 stream).
  - Biases: folded into matmuls via constant-1.0 rows in the zero padding
    (zr1@row657, nh1@row400-of-aligned-view, zr2/nh2@row912) or carried by
    ACT bias (tanh: bin; fc4: sigmoid) / DVE tensor_scalar (fc2/fc3 relu).
  - n-gate: psh=Whn@h+bhn accumulates in PSUM, DVE multiplies r in-place in
    the bank, then Wnx@x accumulates on top (start=False), ACT tanh reads the
    bank with bias=bin. Saves two scalar_tensor_tensor per chunk.
  - h' blend fused as three [128,4,nb] ops (sub/mul/add) writing fp8.
  - fc(t-1) matmul groups are interleaved into tile t's GRU n-chains to keep
    PE busy during the DVE/ACT latency chain (1-tile software pipeline).
"""

import sys

import numpy as np
import ml_dtypes

sys.path.insert(0, "/opt/trn_rl_repo")

import concourse.bacc as bacc
import concourse.mybir as mybir
import concourse.tile as tile
from concourse.bass import ts
from concourse.bass_utils import run_bass_kernel_spmd

BF16 = ml_dtypes.bfloat16
FP8 = ml_dtypes.float8_e4m3

B, F, H, FF = 32768, 257, 400, 600
NCORES = 8
BPC = B // NCORES           # 4096 batch rows per core
NB = 512                    # matmul free-dim tile (one PSUM bank of fp32)
XH1 = 769                   # [x(257) | h1(400) | 1-row(657) | 0] rows
ZRM = 1024                  # [z(400)|pad | r(400)|pad] aligned gate cols

AF = mybir.ActivationFunctionType
ALU = mybir.AluOpType
DR = mybir.MatmulPerfMode.DoubleRow

# packed fp32 bias columns: name -> (offset, n_chunks)
BIAS_LAYOUT = {}
_off = 0
for _n, _c in (("bnx1", 4), ("bnx2", 4), ("bfc2", 5), ("bfc3", 5), ("bfc4", 3)):
    BIAS_LAYOUT[_n] = (_off, _c)
    _off += _c
BIAS_COLS = _off


def _pad2(a, rows, cols):
    out = np.zeros((rows, cols), dtype=np.float64)
    out[: a.shape[0], : a.shape[1]] = a
    return out


def _bias_tile(vec, padded):
    v = np.zeros(padded, dtype=np.float64)
    v[: vec.shape[0]] = vec
    return np.ascontiguousarray(v.reshape(padded // 128, 128).T).astype(np.float32)


def prepare_weights(inp):
    f64 = {k: np.asarray(v, dtype=np.float64) for k, v in inp.items()}
    w = {}

    # fc1 fold for GRU1 input side
    Wx, bx = {}, {}
    for name in ("z", "r", "n"):
        Wx[name] = (f64[f"Wi{name}1"] @ f64["Wfc1"]).T          # [F, H]
        bx[name] = f64[f"bi{name}1"] + f64[f"Wi{name}1"] @ f64["bfc1"]

    # GRU1 z,r: K = [x(257) | h1(400) | 1@657 | x256@658] -> 768 rows
    # (6 chunks), M = [z | pad | r | pad] = 1024 (lane-aligned gates).
    Wzr1 = np.zeros((768, ZRM), dtype=np.float64)
    for g, name in enumerate(("z", "r")):
        c0 = 512 * g
        Wzr1[:F, c0 : c0 + H] = Wx[name]
        Wzr1[F : F + H, c0 : c0 + H] = f64[f"Wh{name}1"].T
        Wzr1[657, c0 : c0 + H] = bx[name] + f64[f"bh{name}1"]
    w["Wzr1"] = Wzr1
    # GRU1 n input side: rhs = xh chunks 0..1 (rows 0..255). x feature 256
    # rides in the nh1 operand instead (xh row 658 = x[:,256] on host).
    w["Wn1x"] = _pad2(Wx["n"][:256], 256, 512)
    # GRU1 n hidden side: rhs = aligned h1 view (xh rows 257..768):
    # pos 400 = 1-row (bhn1), pos 401 = x[:,256] (row 658).
    Wn1h = _pad2(f64["Whn1"].T, 512, 512)
    Wn1h[400, :H] = f64["bhn1"]
    Wn1h[401, :H] = Wx["n"][256]
    w["Wn1h"] = Wn1h

    # GRU2 z,r: K = [g1(512-pad) | h2(400) | 1@912] = 1024 (8 chunks)
    Wzr2 = np.zeros((1024, ZRM), dtype=np.float64)
    for g, name in enumerate(("z", "r")):
        c0 = 512 * g
        Wzr2[:H, c0 : c0 + H] = f64[f"Wi{name}2"].T
        Wzr2[512 : 512 + H, c0 : c0 + H] = f64[f"Wh{name}2"].T
        Wzr2[912, c0 : c0 + H] = f64[f"bi{name}2"] + f64[f"bh{name}2"]
    w["Wzr2"] = Wzr2
    w["Wn2x"] = _pad2(f64["Win2"].T, 512, 512)
    Wn2h = _pad2(f64["Whn2"].T, 512, 512)
    Wn2h[400, :H] = f64["bhn2"]
    w["Wn2h"] = Wn2h

    w["Wfc2"] = _pad2(f64["Wfc2"].T, 512, 640)
    w["Wfc3"] = _pad2(f64["Wfc3"].T, 768, 640)
    w["Wfc4"] = _pad2(f64["Wfc4"].T, 768, 384)

    weights = {k: np.ascontiguousarray(v).astype(FP8) for k, v in w.items()}

    parts = [
        ("bnx1", _bias_tile(bx["n"], 512)),
        ("bnx2", _bias_tile(f64["bin2"], 512)),
        ("bfc2", _bias_tile(f64["bfc2"], 640)),
        ("bfc3", _bias_tile(f64["bfc3"], 640)),
        ("bfc4", _bias_tile(f64["bfc4"], 384)),
    ]
    biases = {"biasT": np.concatenate([p[1] for p in parts], axis=1)}
    return weights, biases


def build_nc(nbt=BPC, nb=NB):
    nc = bacc.Bacc("TRN2", target_bir_lowering=False, debug=False)
    bf = mybir.dt.bfloat16
    f32 = mybir.dt.float32
    f8 = mybir.dt.float8e4

    xh8 = nc.declare_dram_parameter("xh8", [XH1, nbt], f8, isOutput=False)
    h28 = nc.declare_dram_parameter("h28", [512, nbt], f8, isOutput=False)
    h1T = nc.declare_dram_parameter("h1T", [512, nbt], bf, isOutput=False)
    h2T = nc.declare_dram_parameter("h2T", [512, nbt], bf, isOutput=False)
    wd = {}
    for name, k, m in (
        ("Wzr1", 768, ZRM), ("Wn1x", 256, 512), ("Wn1h", 512, 512),
        ("Wzr2", 1024, ZRM), ("Wn2x", 512, 512), ("Wn2h", 512, 512),
        ("Wfc2", 512, 640), ("Wfc3", 768, 640), ("Wfc4", 768, 384),
    ):
        wd[name] = nc.declare_dram_parameter(name, [k, m], f8, isOutput=False)
    biasT_d = nc.declare_dram_parameter("biasT", [128, BIAS_COLS], f32, isOutput=False)
    outT = nc.declare_dram_parameter("outT", [384, nbt], bf, isOutput=True)

    n_tiles = nbt // nb

    with tile.TileContext(nc) as tc:
        with (
            tc.tile_pool(name="wpool", bufs=1) as wpool,
            tc.tile_pool(name="bpool", bufs=1) as bpool,
            tc.tile_pool(name="io", bufs=3) as io,
            tc.tile_pool(name="inp", bufs=3) as inp,
            tc.tile_pool(name="act", bufs=2) as act,
            tc.tile_pool(name="tmp", bufs=2) as tmp,
            tc.tile_pool(name="pzr", bufs=2, space="PSUM") as pzr,
            tc.tile_pool(name="pn", bufs=2, space="PSUM") as pn,
            tc.tile_pool(name="pfc", bufs=2, space="PSUM") as pfc,
        ):
            W, BT = {}, {}

            def load_w(name, eng=None):
                dram = wd[name]
                k, m = dram.shape
                t = wpool.tile([128, k // 128, m], dram.dtype, tag=name)
                r = dram.rearrange("(c p) m -> p c m", p=128)
                (eng or nc.sync).dma_start(out=t, in_=r)
                W[name] = t

            def load_bias():
                biasT = bpool.tile([128, BIAS_COLS], f32, tag="biasT")
                nc.sync.dma_start(out=biasT, in_=biasT_d[:, :])
                for _n, (_o, _c) in BIAS_LAYOUT.items():
                    BT[_n] = biasT[:, _o : _o + _c]

            xh_zr = xh8[0:768, :].rearrange("(c p) n -> p c n", p=128)
            h1m_al = xh8[257 : 257 + 512, :].rearrange("(c p) n -> p c n", p=128)
            h28_al = h28.rearrange("(c p) n -> p c n", p=128)
            h1_bl = h1T.rearrange("(c p) n -> p c n", p=128)
            h2_bl = h2T.rearrange("(c p) n -> p c n", p=128)
            outT_r = outT.rearrange("(c p) n -> p c n", p=128)

            def load_inputs(t):
                sl = ts(t, nb)
                xh = inp.tile([128, 6, nb], f8, tag="xh")      # zr1 K (nx1: c0..3)
                nc.sync.dma_start(out=xh, in_=xh_zr[:, :, sl])
                h1m = inp.tile([128, 4, nb], f8, tag="h1m")    # nh1 rhs (aligned h1)
                nc.sync.dma_start(out=h1m, in_=h1m_al[:, :, sl])
                h1s = inp.tile([128, 4, nb], bf, tag="h1s")    # blend h1
                nc.sync.dma_start(out=h1s, in_=h1_bl[:, :, sl])
                h2s = inp.tile([128, 4, nb], bf, tag="h2s")    # blend h2
                nc.sync.dma_start(out=h2s, in_=h2_bl[:, :, sl])
                # GRU2 K operand [g1(512) | h2(512)] as two tiles so the h2
                # half's matmuls don't wait on GRU1's blend.
                zr2h = inp.tile([128, 4, nb], f8, tag="zr2h")
                nc.sync.dma_start(out=zr2h, in_=h28_al[:, :, sl])
                zr2g = io.tile([128, 4, nb], f8, tag="zr2g")
                return xh, h1m, h1s, h2s, zr2g, zr2h

            def matseq_dr(ps, Wt, col, segs, start0=True):
                """Accumulate into ps via DoubleRow passes. segs: list of
                (w_chunk0, rhs_tile, rhs_chunk0, n_pairs)."""
                total = sum(s[3] for s in segs)
                i = 0
                for wk0, rhs_t, rk0, npairs in segs:
                    for j in range(npairs):
                        wk, rk = wk0 + 2 * j, rk0 + 2 * j
                        nc.tensor.matmul(
                            ps, Wt[:, wk : wk + 2, col : col + 128],
                            rhs_t[:, rk : rk + 2, :],
                            start=(start0 and i == 0), stop=(i == total - 1),
                            perf_mode=DR,
                            skip_group_check=(not start0))
                        i += 1

            def zr_items(Wzr, segs, zro):
                """8 gate chunks as 4 pair-group closures, r pairs first."""
                def item(p):
                    def run():
                        ps = pzr.tile([128, 2, nb], f32, tag="ps_zr")
                        for i in (0, 1):
                            matseq_dr(ps[:, i, :], Wzr, (2 * p + i) * 128, segs)
                        nc.scalar.activation(zro[:, 2 * p : 2 * p + 2, :], ps,
                                             AF.Sigmoid)
                    return run
                return [item(p) for p in (2, 3, 0, 1)]

            def n_chain(Wnh, nh_segs, Wnx, nx_segs, zro, bnx, n_t, filler):
                """n = tanh(Wnx@x + bnx + r*(Wnh@h + bhn)); r = zro[:,4+c,:].
                Runs filler() work items between chunks to keep PE fed."""
                for c in range(4):
                    ps = pn.tile([128, nb], f32, tag="ps_n")
                    matseq_dr(ps, Wnh, c * 128, nh_segs)
                    nc.vector.tensor_mul(ps, ps, zro[:, 4 + c, :])
                    filler()
                    matseq_dr(ps, Wnx, c * 128, nx_segs, start0=False)
                    nc.scalar.activation(n_t[:, c, :], ps, AF.Tanh,
                                         bias=bnx[:, c : c + 1])
                    filler()

            def blend(n_t, h_s, zro, out_t, tag):
                # two chunk-pair halves -> consumers of half 0 unblock before
                # the second half's tanh has even finished
                for h in (0, 1):
                    sl = slice(2 * h, 2 * h + 2)
                    d = tmp.tile([128, 2, nb], bf, tag=f"d{tag}{h}")
                    nc.vector.tensor_sub(d, h_s[:, sl, :], n_t[:, sl, :])
                    zd = tmp.tile([128, 2, nb], bf, tag=f"zd{tag}{h}")
                    nc.vector.tensor_mul(zd, zro[:, sl, :], d)
                    nc.vector.tensor_add(out_t[:, sl, :], n_t[:, sl, :], zd)

            def fc_items(g2, f2, f3, o):
                """List of closures: one PSUM group + activation each."""
                items = []

                def fc_item(Wt, kc, rhs, m, kind, out_ap, bias):
                    def run():
                        ps = pfc.tile([128, nb], f32, tag="ps_fc")
                        matseq_dr(ps, Wt, m * 128, [(0, rhs, 0, kc // 2)])
                        if kind == "relu":
                            nc.vector.tensor_scalar(
                                out_ap, ps, bias, 0.0, op0=ALU.add, op1=ALU.max)
                        else:
                            nc.scalar.activation(out_ap, ps, AF.Sigmoid,
                                                 bias=bias)
                    return run

                for m in range(5):
                    items.append(fc_item(W["Wfc2"], 4, g2, m, "relu",
                                         f2[:, m, :], BT["bfc2"][:, m : m + 1]))
                for m in range(5):
                    items.append(fc_item(W["Wfc3"], 6, f2, m, "relu",
                                         f3[:, m, :], BT["bfc3"][:, m : m + 1]))
                for m in range(3):
                    items.append(fc_item(W["Wfc4"], 6, f3, m, "sig",
                                         o[:, m, :], BT["bfc4"][:, m : m + 1]))
                return items

            # Startup staging: the DMA hw round-robins ALL outstanding
            # transfers, so the first-needed data must be the ONLY data in
            # flight. Stage 1 (ungated): everything GRU1(0) needs. Later
            # stages are held back by tiny gate reads on their issue rings —
            # an in-order ring can't issue its next DMA until the gate's
            # input tile has fully landed.
            load_w("Wzr1", eng=nc.scalar)        # stage 1
            ins0 = load_inputs(0)                # stage 1 (sync ring)
            load_w("Wn1h", eng=nc.gpsimd)        # stage 1
            load_w("Wn1x", eng=nc.gpsimd)        # stage 1
            # bias DMA is 128 tiny descriptors (~7us of ring issue, ~no
            # bandwidth): trickle it on the sync ring behind tile0's inputs
            load_bias()
            # ACT-table warmup (sigmoid+tanh) before the first gate sigmoid
            warm = bpool.tile([128, 1], f32, tag="warm")
            nc.vector.memset(warm, 0.0)
            nc.scalar.activation(warm, warm, AF.Sigmoid)
            nc.scalar.activation(warm, warm, AF.Tanh)

            gate = bpool.tile([128, 8], f32, tag="gate")

            def ring_gate(eng_act, col, tiles):
                for j, tl in enumerate(tiles):
                    src = tl[0:1, 0, 0:1] if len(tl.shape) == 3 else tl[0:1, 0:1]
                    if eng_act is nc.scalar:
                        nc.scalar.activation(gate[0:1, col + j : col + j + 1],
                                             src, AF.Copy)
                    else:
                        nc.gpsimd.tensor_scalar_add(
                            gate[0:1, col + j : col + j + 1], src, 0.0)

            # stage 2 (gated on stage 1): GRU2-side data
            xh0, h1m0 = ins0[0], ins0[1]
            ring_gate(nc.scalar, 0, (xh0, W["Wzr1"]))
            load_w("Wzr2", eng=nc.scalar)
            ring_gate(nc.gpsimd, 2, (h1m0, W["Wn1h"]))
            for name in ("Wn2x", "Wn2h"):
                load_w(name, eng=nc.gpsimd)
            # stage 3 (gated on stage 2): fc weights, needed ~35us in
            ring_gate(nc.gpsimd, 4, (W["Wzr2"],))
            for name in ("Wfc2", "Wfc3", "Wfc4"):
                load_w(name, eng=nc.gpsimd)

            fcq = []            # fc work items from tile t-1
            zr1q = []           # prefetched zr1(t) pair items
            prev_out = None     # (o_tile, t-1) awaiting DMA out
            tiles_in = {0: ins0}

            for t in range(n_tiles):
                xh, h1m, h1s, h2s, zr2g, zr2h = tiles_in.pop(t)
                if t + 1 < n_tiles:     # prefetch next tile's inputs now
                    tiles_in[t + 1] = load_inputs(t + 1)

                def fc_fill(k=2):
                    for _ in range(k):
                        if fcq:
                            fcq.pop(0)()

                # ---- GRU1 ----
                if t == 0:
                    zro1 = act.tile([128, 8, nb], bf, tag="zro1")
                    for it in zr_items(W["Wzr1"], [(0, xh, 0, 3)], zro1):
                        it()
                else:
                    assert not zr1q  # emitted during n_chain2(t-1)
                    zro1 = zro1_next  # noqa: F821
                n1 = act.tile([128, 4, nb], bf, tag="n1")
                n_chain(W["Wn1h"], [(0, h1m, 0, 2)], W["Wn1x"],
                        [(0, xh, 0, 1)], zro1, BT["bnx1"], n1, fc_fill)
                blend(n1, h1s, zro1, zr2g, "1")

                # ---- GRU2 zr: h2-half passes first (independent of blend1),
                # leftover fc(t-1) between pair groups ----
                zro2 = act.tile([128, 8, nb], bf, tag="zro2")
                zr2_segs = [(4, zr2h, 0, 2), (0, zr2g, 0, 2)]
                for it in zr_items(W["Wzr2"], zr2_segs, zro2):
                    it()
                    fc_fill(1)
                fc_fill(len(fcq))

                if t + 1 < n_tiles:
                    zro1_next = act.tile([128, 8, nb], bf, tag="zro1")
                    zr1q = zr_items(W["Wzr1"], [(0, tiles_in[t + 1][0], 0, 3)],
                                    zro1_next)

                def zr1_fill():
                    if zr1q:
                        zr1q.pop(0)()

                # ---- GRU2 n-chain, zr1(t+1) interleaved as PE filler ----
                g2 = io.tile([128, 4, nb], f8, tag="g2")
                n2 = act.tile([128, 4, nb], bf, tag="n2")
                n_chain(W["Wn2h"], [(0, zr2h, 0, 2)], W["Wn2x"],
                        [(0, zr2g, 0, 2)], zro2, BT["bnx2"], n2, zr1_fill)
                while zr1q:
                    zr1q.pop(0)()
                blend(n2, h2s, zro2, g2, "2")

                if prev_out is not None:
                    o_prev, tp = prev_out
                    nc.sync.dma_start(out=outT_r[:, :, ts(tp, nb)], in_=o_prev)

                # ---- queue this tile's fc stage ----
                f2 = io.tile([128, 6, nb], f8, tag="f2")
                f3 = io.tile([128, 6, nb], f8, tag="f3")
                if t < 3:   # io pool bufs=3: zero the K-pad chunk once per buf
                    nc.gpsimd.memset(f2[:, 5, :], 0.0)
                    nc.gpsimd.memset(f3[:, 5, :], 0.0)
                o = io.tile([128, 3, nb], bf, tag="o")
                fcq = fc_items(g2, f2, f3, o)
                prev_out = (o, t)

            while fcq:
                fcq.pop(0)()
            o_last, tl = prev_out
            nc.sync.dma_start(out=outT_r[:, :, ts(tl, nb)], in_=o_last)

    nc.compile()
    return nc


def _shard_inputs(inp, weights, biases):
    x = np.asarray(inp["x"], dtype=np.float32)
    h1 = np.asarray(inp["h1"], dtype=np.float32)
    h2 = np.asarray(inp["h2"], dtype=np.float32)

    xh8 = np.zeros((NCORES, XH1, BPC), dtype=FP8)
    h28 = np.zeros((NCORES, 512, BPC), dtype=FP8)
    h1T = np.zeros((NCORES, 512, BPC), dtype=BF16)
    h2T = np.zeros((NCORES, 512, BPC), dtype=BF16)
    for i in range(NCORES):
        sl = slice(i * BPC, (i + 1) * BPC)
        xh8[i, :F] = x[sl].T.astype(FP8)
        xh8[i, F : F + H] = h1[sl].T.astype(FP8)
        xh8[i, 657] = 1.0
        xh8[i, 658] = x[sl, 256].astype(FP8)  # x feat 256 rides in nh1's K
        h28[i, :H] = h2[sl].T.astype(FP8)
        h28[i, 400] = 1.0
        h1T[i, :H] = h1[sl].T.astype(BF16)
        h2T[i, :H] = h2[sl].T.astype(BF16)

    in_maps = []
    for i in range(NCORES):
        m = {"xh8": xh8[i], "h28": h28[i], "h1T": h1T[i], "h2T": h2T[i]}
        m.update(weights)
        m.update(biases)
        in_maps.append(m)
    return in_maps


def _run(inp, trace=False):
    weights, biases = prepare_weights(inp)
    nc = build_nc()
    in_maps = _shard_inputs(inp, weights, biases)
    res = run_bass_kernel_spmd(nc, in_maps, list(range(NCORES)), trace=trace)
    out = np.empty((B, F), dtype=np.float32)
    for i in range(NCORES):
        out[i * BPC : (i + 1) * BPC] = (
            np.asarray(res.results[i]["outT"][:F]).astype(np.float32).T
        )
    return out, res


def kernel(**inputs) -> np.ndarray:
    out, _ = _run(inputs, trace=False)
    return out


# revision 28
# speedup vs baseline: 1.1832x; 1.1832x over previous
"""NsNet2 single-step (fc1 + 2x GRU cell + 3x FC) Trainium2 kernel, v2.

Strategy (v2 — all-fp8 DoubleRow):
  - Pure data parallel: B=32768 -> 4096 rows/core on 8 cores; feature-major
    on chip ([feat, batch]) so matmul operands need no on-chip transposes.
  - EVERY matmul is fp8e4m3 + DoubleRow (0.5 cyc/row) with even K-chunk
    counts (K pads to 2-chunk multiples); numpy-sim predicts rel_err 1.6e-2.
  - Gate layout: zr weights have M=1024 = [z(400)|pad|r(400)|pad] so both
    gates land lane-aligned -> no r-realign DMAs. r-pairs are computed first
    so the n-gate chain can start early.
  - GRU2 contraction operand is one aligned block zr2op = [g1(512)|h2(512)]:
    chunks 0..3 written by GRU1's blend (fp8), chunks 4..7 DMA'd from padded
    h2; nh2/nx2 are chunk-slices of it (no separate h2 fp8 stream).
  - Biases: folded into matmuls via constant-1.0 rows in the zero padding
    (zr1@row657, nh1@row400-of-aligned-view, zr2/nh2@row912) or carried by
    ACT bias (tanh: bin; fc4: sigmoid) / DVE tensor_scalar (fc2/fc3 relu).
  - n-gate: psh=Whn@h+bhn accumulates in PSUM, DVE multiplies r in-place in
    the bank, then Wnx@x accumulates on top (start=False), ACT tanh reads the
    bank with bias=bin. Saves two scalar_tensor_tensor per chunk.
  - h' blend fused as three [128,4,nb] ops (sub/mul/add) writing fp8.
  - fc(t-1) matmul groups are interleaved into tile t's GRU n-chains to keep
    PE busy during the DVE/ACT latency chain (1-tile software pipeline).
"""

import sys

import numpy as np
import ml_dtypes

sys.path.insert(0, "/opt/trn_rl_repo")

import concourse.bacc as bacc
import concourse.mybir as mybir
import concourse.tile as tile
from concourse.bass import ts
from concourse.bass_utils import run_bass_kernel_spmd

BF16 = ml_dtypes.bfloat16
FP8 = ml_dtypes.float8_e4m3

B, F, H, FF = 32768, 257, 400, 600
NCORES = 8
BPC = B // NCORES           # 4096 batch rows per core
NB = 512                    # matmul free-dim tile (one PSUM bank of fp32)
XH1 = 769                   # [x(257) | h1(400) | 1-row(657) | 0] rows
ZRM = 1024                  # [z(400)|pad | r(400)|pad] aligned gate cols

AF = mybir.ActivationFunctionType
ALU = mybir.AluOpType
DR = mybir.MatmulPerfMode.DoubleRow

# packed fp32 bias columns: name -> (offset, n_chunks)
BIAS_LAYOUT = {}
_off = 0
for _n, _c in (("bnx1", 4), ("bnx2", 4), ("bfc2", 5), ("bfc3", 5), ("bfc4", 3)):
    BIAS_LAYOUT[_n] = (_off, _c)
    _off += _c
BIAS_COLS = _off


def _pad2(a, rows, cols):
    out = np.zeros((rows, cols), dtype=np.float64)
    out[: a.shape[0], : a.shape[1]] = a
    return out


def _bias_tile(vec, padded):
    v = np.zeros(padded, dtype=np.float64)
    v[: vec.shape[0]] = vec
    return np.ascontiguousarray(v.reshape(padded // 128, 128).T).astype(np.float32)


def prepare_weights(inp):
    f64 = {k: np.asarray(v, dtype=np.float64) for k, v in inp.items()}
    w = {}

    # fc1 fold for GRU1 input side
    Wx, bx = {}, {}
    for name in ("z", "r", "n"):
        Wx[name] = (f64[f"Wi{name}1"] @ f64["Wfc1"]).T          # [F, H]
        bx[name] = f64[f"bi{name}1"] + f64[f"Wi{name}1"] @ f64["bfc1"]

    # GRU1 z,r: K = [x(257) | h1(400) | 1@657 | x256@658] -> 768 rows
    # (6 chunks), M = [z | pad | r | pad] = 1024 (lane-aligned gates).
    Wzr1 = np.zeros((768, ZRM), dtype=np.float64)
    for g, name in enumerate(("z", "r")):
        c0 = 512 * g
        Wzr1[:F, c0 : c0 + H] = Wx[name]
        Wzr1[F : F + H, c0 : c0 + H] = f64[f"Wh{name}1"].T
        Wzr1[657, c0 : c0 + H] = bx[name] + f64[f"bh{name}1"]
    w["Wzr1"] = Wzr1
    # GRU1 n input side: rhs = xh chunks 0..1 (rows 0..255). x feature 256
    # rides in the nh1 operand instead (xh row 658 = x[:,256] on host).
    w["Wn1x"] = _pad2(Wx["n"][:256], 256, 512)
    # GRU1 n hidden side: rhs = aligned h1 view (xh rows 257..768):
    # pos 400 = 1-row (bhn1), pos 401 = x[:,256] (row 658).
    Wn1h = _pad2(f64["Whn1"].T, 512, 512)
    Wn1h[400, :H] = f64["bhn1"]
    Wn1h[401, :H] = Wx["n"][256]
    w["Wn1h"] = Wn1h

    # GRU2 z,r: K = [g1(512-pad) | h2(400) | 1@912] = 1024 (8 chunks)
    Wzr2 = np.zeros((1024, ZRM), dtype=np.float64)
    for g, name in enumerate(("z", "r")):
        c0 = 512 * g
        Wzr2[:H, c0 : c0 + H] = f64[f"Wi{name}2"].T
        Wzr2[512 : 512 + H, c0 : c0 + H] = f64[f"Wh{name}2"].T
        Wzr2[912, c0 : c0 + H] = f64[f"bi{name}2"] + f64[f"bh{name}2"]
    w["Wzr2"] = Wzr2
    w["Wn2x"] = _pad2(f64["Win2"].T, 512, 512)
    Wn2h = _pad2(f64["Whn2"].T, 512, 512)
    Wn2h[400, :H] = f64["bhn2"]
    w["Wn2h"] = Wn2h

    w["Wfc2"] = _pad2(f64["Wfc2"].T, 512, 640)
    w["Wfc3"] = _pad2(f64["Wfc3"].T, 768, 640)
    w["Wfc4"] = _pad2(f64["Wfc4"].T, 768, 384)

    weights = {k: np.ascontiguousarray(v).astype(FP8) for k, v in w.items()}

    parts = [
        ("bnx1", _bias_tile(bx["n"], 512)),
        ("bnx2", _bias_tile(f64["bin2"], 512)),
        ("bfc2", _bias_tile(f64["bfc2"], 640)),
        ("bfc3", _bias_tile(f64["bfc3"], 640)),
        ("bfc4", _bias_tile(f64["bfc4"], 384)),
    ]
    biases = {"biasT": np.concatenate([p[1] for p in parts], axis=1)}
    return weights, biases


def build_nc(nbt=BPC, nb=NB):
    nc = bacc.Bacc("TRN2", target_bir_lowering=False, debug=False)
    bf = mybir.dt.bfloat16
    f32 = mybir.dt.float32
    f8 = mybir.dt.float8e4

    xh8 = nc.declare_dram_parameter("xh8", [XH1, nbt], f8, isOutput=False)
    h28 = nc.declare_dram_parameter("h28", [512, nbt], f8, isOutput=False)
    h1T = nc.declare_dram_parameter("h1T", [512, nbt], bf, isOutput=False)
    h2T = nc.declare_dram_parameter("h2T", [512, nbt], bf, isOutput=False)
    wd = {}
    for name, k, m in (
        ("Wzr1", 768, ZRM), ("Wn1x", 256, 512), ("Wn1h", 512, 512),
        ("Wzr2", 1024, ZRM), ("Wn2x", 512, 512), ("Wn2h", 512, 512),
        ("Wfc2", 512, 640), ("Wfc3", 768, 640), ("Wfc4", 768, 384),
    ):
        wd[name] = nc.declare_dram_parameter(name, [k, m], f8, isOutput=False)
    biasT_d = nc.declare_dram_parameter("biasT", [128, BIAS_COLS], f32, isOutput=False)
    outT = nc.declare_dram_parameter("outT", [384, nbt], bf, isOutput=True)

    n_tiles = nbt // nb

    with tile.TileContext(nc) as tc:
        with (
            tc.tile_pool(name="wpool", bufs=1) as wpool,
            tc.tile_pool(name="bpool", bufs=1) as bpool,
            tc.tile_pool(name="io", bufs=3) as io,
            tc.tile_pool(name="inp", bufs=3) as inp,
            tc.tile_pool(name="act", bufs=2) as act,
            tc.tile_pool(name="tmp", bufs=2) as tmp,
            tc.tile_pool(name="pzr", bufs=2, space="PSUM") as pzr,
            tc.tile_pool(name="pn", bufs=2, space="PSUM") as pn,
            tc.tile_pool(name="pfc", bufs=2, space="PSUM") as pfc,
        ):
            W, BT = {}, {}

            def load_w(name, eng=None):
                dram = wd[name]
                k, m = dram.shape
                t = wpool.tile([128, k // 128, m], dram.dtype, tag=name)
                r = dram.rearrange("(c p) m -> p c m", p=128)
                (eng or nc.sync).dma_start(out=t, in_=r)
                W[name] = t

            def load_bias():
                biasT = bpool.tile([128, BIAS_COLS], f32, tag="biasT")
                nc.sync.dma_start(out=biasT, in_=biasT_d[:, :])
                for _n, (_o, _c) in BIAS_LAYOUT.items():
                    BT[_n] = biasT[:, _o : _o + _c]

            xh_zr = xh8[0:768, :].rearrange("(c p) n -> p c n", p=128)
            h1m_al = xh8[257 : 257 + 512, :].rearrange("(c p) n -> p c n", p=128)
            h28_al = h28.rearrange("(c p) n -> p c n", p=128)
            h1_bl = h1T.rearrange("(c p) n -> p c n", p=128)
            h2_bl = h2T.rearrange("(c p) n -> p c n", p=128)
            outT_r = outT.rearrange("(c p) n -> p c n", p=128)

            def load_inputs(t, eng=None, crit_only=False, rest=None):
                """crit_only: issue just xh+h1m (stage 1), return partial;
                rest: finish a partial load on ring `eng`."""
                sl = ts(t, nb)
                e = eng or nc.sync
                if rest is None:
                    xh = inp.tile([128, 6, nb], f8, tag="xh")  # zr1 K
                    e.dma_start(out=xh, in_=xh_zr[:, :, sl])
                    h1m = inp.tile([128, 4, nb], f8, tag="h1m")  # nh1 rhs
                    e.dma_start(out=h1m, in_=h1m_al[:, :, sl])
                    if crit_only:
                        return [xh, h1m]
                else:
                    xh, h1m = rest
                h1s = inp.tile([128, 4, nb], bf, tag="h1s")    # blend h1
                e.dma_start(out=h1s, in_=h1_bl[:, :, sl])
                h2s = inp.tile([128, 4, nb], bf, tag="h2s")    # blend h2
                e.dma_start(out=h2s, in_=h2_bl[:, :, sl])
                # GRU2 K operand [g1(512) | h2(512)] as two tiles so the h2
                # half's matmuls don't wait on GRU1's blend.
                zr2h = inp.tile([128, 4, nb], f8, tag="zr2h")
                e.dma_start(out=zr2h, in_=h28_al[:, :, sl])
                zr2g = io.tile([128, 4, nb], f8, tag="zr2g")
                return xh, h1m, h1s, h2s, zr2g, zr2h

            def matseq_dr(ps, Wt, col, segs, start0=True):
                """Accumulate into ps via DoubleRow passes. segs: list of
                (w_chunk0, rhs_tile, rhs_chunk0, n_pairs)."""
                total = sum(s[3] for s in segs)
                i = 0
                for wk0, rhs_t, rk0, npairs in segs:
                    for j in range(npairs):
                        wk, rk = wk0 + 2 * j, rk0 + 2 * j
                        nc.tensor.matmul(
                            ps, Wt[:, wk : wk + 2, col : col + 128],
                            rhs_t[:, rk : rk + 2, :],
                            start=(start0 and i == 0), stop=(i == total - 1),
                            perf_mode=DR,
                            skip_group_check=(not start0))
                        i += 1

            def zr_items(Wzr, segs, zro):
                """8 gate chunks as 4 pair-group closures, r pairs first."""
                def item(p):
                    def run():
                        ps = pzr.tile([128, 2, nb], f32, tag="ps_zr")
                        for i in (0, 1):
                            matseq_dr(ps[:, i, :], Wzr, (2 * p + i) * 128, segs)
                        nc.scalar.activation(zro[:, 2 * p : 2 * p + 2, :], ps,
                                             AF.Sigmoid)
                    return run
                return [item(p) for p in (2, 3, 0, 1)]

            def n_chain(Wnh, nh_segs, Wnx, nx_segs, zro, bnx, n_t, filler):
                """n = tanh(Wnx@x + bnx + r*(Wnh@h + bhn)); r = zro[:,4+c,:].
                Runs filler() work items between chunks to keep PE fed."""
                for c in range(4):
                    ps = pn.tile([128, nb], f32, tag="ps_n")
                    matseq_dr(ps, Wnh, c * 128, nh_segs)
                    nc.vector.tensor_mul(ps, ps, zro[:, 4 + c, :])
                    filler()
                    matseq_dr(ps, Wnx, c * 128, nx_segs, start0=False)
                    nc.scalar.activation(n_t[:, c, :], ps, AF.Tanh,
                                         bias=bnx[:, c : c + 1])
                    filler()

            def blend(n_t, h_s, zro, out_t, tag):
                # two chunk-pair halves -> consumers of half 0 unblock before
                # the second half's tanh has even finished
                for h in (0, 1):
                    sl = slice(2 * h, 2 * h + 2)
                    d = tmp.tile([128, 2, nb], bf, tag=f"d{tag}{h}")
                    nc.vector.tensor_sub(d, h_s[:, sl, :], n_t[:, sl, :])
                    zd = tmp.tile([128, 2, nb], bf, tag=f"zd{tag}{h}")
                    nc.vector.tensor_mul(zd, zro[:, sl, :], d)
                    nc.vector.tensor_add(out_t[:, sl, :], n_t[:, sl, :], zd)

            def fc_items(g2, f2, f3, o):
                """List of closures: one PSUM group + activation each."""
                items = []

                def fc_item(Wt, kc, rhs, m, kind, out_ap, bias):
                    def run():
                        ps = pfc.tile([128, nb], f32, tag="ps_fc")
                        matseq_dr(ps, Wt, m * 128, [(0, rhs, 0, kc // 2)])
                        if kind == "relu":
                            nc.vector.tensor_scalar(
                                out_ap, ps, bias, 0.0, op0=ALU.add, op1=ALU.max)
                        else:
                            nc.scalar.activation(out_ap, ps, AF.Sigmoid,
                                                 bias=bias)
                    return run

                for m in range(5):
                    items.append(fc_item(W["Wfc2"], 4, g2, m, "relu",
                                         f2[:, m, :], BT["bfc2"][:, m : m + 1]))
                for m in range(5):
                    items.append(fc_item(W["Wfc3"], 6, f2, m, "relu",
                                         f3[:, m, :], BT["bfc3"][:, m : m + 1]))
                for m in range(3):
                    items.append(fc_item(W["Wfc4"], 6, f3, m, "sig",
                                         o[:, m, :], BT["bfc4"][:, m : m + 1]))
                return items

            # Startup staging: the DMA hw round-robins ALL outstanding
            # transfers, so the first-needed data must be the ONLY data in
            # flight. Stage 1 (ungated): everything GRU1(0) needs. Later
            # stages are held back by tiny gate reads on their issue rings —
            # an in-order ring can't issue its next DMA until the gate's
            # input tile has fully landed.
            # Stage 1 — ONLY what GRU1(0) needs up to the n-chain: xh, h1m,
            # Wzr1, Wn1h, Wn1x (~1.7MB -> all landed by ~5us).
            load_w("Wzr1", eng=nc.scalar)
            crit0 = load_inputs(0, crit_only=True)        # sync: xh, h1m
            load_w("Wn1h", eng=nc.gpsimd)
            load_w("Wn1x", eng=nc.gpsimd)
            # ACT-table warmup (sigmoid+tanh) before the first gate sigmoid
            warm = bpool.tile([128, 1], f32, tag="warm")
            nc.vector.memset(warm, 0.0)
            nc.scalar.activation(warm, warm, AF.Sigmoid)
            nc.scalar.activation(warm, warm, AF.Tanh)

            gate = bpool.tile([128, 8], f32, tag="gate")

            def ring_gate(eng_act, col, tiles):
                for j, tl in enumerate(tiles):
                    src = tl[0:1, 0, 0:1] if len(tl.shape) == 3 else tl[0:1, 0:1]
                    if eng_act is nc.scalar:
                        nc.scalar.activation(gate[0:1, col + j : col + j + 1],
                                             src, AF.Copy)
                    else:
                        nc.gpsimd.tensor_scalar_add(
                            gate[0:1, col + j : col + j + 1], src, 0.0)

            # stage 2 (gated on stage 1): Wzr2 on scalar; bias (128 tiny
            # descriptors, ~7us ring issue but ~no bandwidth), rest of
            # tile0's inputs, n2 weights on gpsimd
            ring_gate(nc.scalar, 0, (crit0[0], W["Wzr1"]))
            load_w("Wzr2", eng=nc.scalar)
            ring_gate(nc.gpsimd, 2, (crit0[1], W["Wn1h"]))
            load_bias()
            ins0 = load_inputs(0, eng=nc.gpsimd, rest=crit0)
            for name in ("Wn2x", "Wn2h"):
                load_w(name, eng=nc.gpsimd)
            # stage 3 (gated on stage 2): fc weights (needed ~35us in), then
            # tile 1's inputs
            ring_gate(nc.gpsimd, 4, (W["Wzr2"],))
            for name in ("Wfc2", "Wfc3", "Wfc4"):
                load_w(name, eng=nc.gpsimd)

            fcq = []            # fc work items from tile t-1
            zr1q = []           # prefetched zr1(t) pair items
            prev_out = None     # (o_tile, t-1) awaiting DMA out
            tiles_in = {0: ins0}

            for t in range(n_tiles):
                xh, h1m, h1s, h2s, zr2g, zr2h = tiles_in.pop(t)
                if t + 1 < n_tiles:     # prefetch next tile's inputs now
                    # tile1's prefetch rides the gated gpsimd ring so it
                    # can't crowd out the startup stages
                    tiles_in[t + 1] = load_inputs(
                        t + 1, eng=nc.gpsimd if t == 0 else None)

                def fc_fill(k=2):
                    for _ in range(k):
                        if fcq:
                            fcq.pop(0)()

                # ---- GRU1 ----
                if t == 0:
                    zro1 = act.tile([128, 8, nb], bf, tag="zro1")
                    for it in zr_items(W["Wzr1"], [(0, xh, 0, 3)], zro1):
                        it()
                else:
                    assert not zr1q  # emitted during n_chain2(t-1)
                    zro1 = zro1_next  # noqa: F821
                n1 = act.tile([128, 4, nb], bf, tag="n1")
                n_chain(W["Wn1h"], [(0, h1m, 0, 2)], W["Wn1x"],
                        [(0, xh, 0, 1)], zro1, BT["bnx1"], n1, fc_fill)
                blend(n1, h1s, zro1, zr2g, "1")

                # ---- GRU2 zr: h2-half passes first (independent of blend1),
                # leftover fc(t-1) between pair groups ----
                zro2 = act.tile([128, 8, nb], bf, tag="zro2")
                zr2_segs = [(4, zr2h, 0, 2), (0, zr2g, 0, 2)]
                for it in zr_items(W["Wzr2"], zr2_segs, zro2):
                    it()
                    fc_fill(1)
                fc_fill(len(fcq))

                if t + 1 < n_tiles:
                    zro1_next = act.tile([128, 8, nb], bf, tag="zro1")
                    zr1q = zr_items(W["Wzr1"], [(0, tiles_in[t + 1][0], 0, 3)],
                                    zro1_next)

                def zr1_fill():
                    if zr1q:
                        zr1q.pop(0)()

                # ---- GRU2 n-chain, zr1(t+1) interleaved as PE filler ----
                g2 = io.tile([128, 4, nb], f8, tag="g2")
                n2 = act.tile([128, 4, nb], bf, tag="n2")
                n_chain(W["Wn2h"], [(0, zr2h, 0, 2)], W["Wn2x"],
                        [(0, zr2g, 0, 2)], zro2, BT["bnx2"], n2, zr1_fill)
                while zr1q:
                    zr1q.pop(0)()
                blend(n2, h2s, zro2, g2, "2")

                if prev_out is not None:
                    o_prev, tp = prev_out
                    nc.sync.dma_start(out=outT_r[:, :, ts(tp, nb)], in_=o_prev)

                # ---- queue this tile's fc stage ----
                f2 = io.tile([128, 6, nb], f8, tag="f2")
                f3 = io.tile([128, 6, nb], f8, tag="f3")
                if t < 3:   # io pool bufs=3: zero the K-pad chunk once per buf
                    nc.gpsimd.memset(f2[:, 5, :], 0.0)
                    nc.gpsimd.memset(f3[:, 5, :], 0.0)
                o = io.tile([128, 3, nb], bf, tag="o")
                fcq = fc_items(g2, f2, f3, o)
                prev_out = (o, t)

            while fcq:
                fcq.pop(0)()
            o_last, tl = prev_out
            nc.sync.dma_start(out=outT_r[:, :, ts(tl, nb)], in_=o_last)

    nc.compile()
    return nc


def _shard_inputs(inp, weights, biases):
    x = np.asarray(inp["x"], dtype=np.float32)
    h1 = np.asarray(inp["h1"], dtype=np.float32)
    h2 = np.asarray(inp["h2"], dtype=np.float32)

    xh8 = np.zeros((NCORES, XH1, BPC), dtype=FP8)
    h28 = np.zeros((NCORES, 512, BPC), dtype=FP8)
    h1T = np.zeros((NCORES, 512, BPC), dtype=BF16)
    h2T = np.zeros((NCORES, 512, BPC), dtype=BF16)
    for i in range(NCORES):
        sl = slice(i * BPC, (i + 1) * BPC)
        xh8[i, :F] = x[sl].T.astype(FP8)
        xh8[i, F : F + H] = h1[sl].T.astype(FP8)
        xh8[i, 657] = 1.0
        xh8[i, 658] = x[sl, 256].astype(FP8)  # x feat 256 rides in nh1's K
        h28[i, :H] = h2[sl].T.astype(FP8)
        h28[i, 400] = 1.0
        h1T[i, :H] = h1[sl].T.astype(BF16)
        h2T[i, :H] = h2[sl].T.astype(BF16)

    in_maps = []
    for i in range(NCORES):
        m = {"xh8": xh8[i], "h28": h28[i], "h1T": h1T[i], "h2T": h2T[i]}
        m.update(weights)
        m.update(biases)
        in_maps.append(m)
    return in_maps


def _run(inp, trace=False):
    weights, biases = prepare_weights(inp)
    nc = build_nc()
    in_maps = _shard_inputs(inp, weights, biases)
    res = run_bass_kernel_spmd(nc, in_maps, list(range(NCORES)), trace=trace)
    out = np.empty((B, F), dtype=np.float32)
    for i in range(NCORES):
        out[i * BPC : (i + 1) * BPC] = (
            np.asarray(res.results[i]["outT"][:F]).astype(np.float32).T
        )
    return out, res


def kernel(**inputs) -> np.ndarray:
    out, _ = _run(inputs, trace=False)
    return out


# revision 36
# speedup vs baseline: 1.2426x; 1.0502x over previous
"""NsNet2 single-step (fc1 + 2x GRU cell + 3x FC) Trainium2 kernel, v2.

Strategy (v2 — all-fp8 DoubleRow):
  - Pure data parallel: B=32768 -> 4096 rows/core on 8 cores; feature-major
    on chip ([feat, batch]) so matmul operands need no on-chip transposes.
  - EVERY matmul is fp8e4m3 + DoubleRow (0.5 cyc/row) with even K-chunk
    counts (K pads to 2-chunk multiples); numpy-sim predicts rel_err 1.6e-2.
  - Gate layout: zr weights have M=1024 = [z(400)|pad|r(400)|pad] so both
    gates land lane-aligned -> no r-realign DMAs. r-pairs are computed first
    so the n-gate chain can start early.
  - GRU2 contraction operand is one aligned block zr2op = [g1(512)|h2(512)]:
    chunks 0..3 written by GRU1's blend (fp8), chunks 4..7 DMA'd from padded
    h2; nh2/nx2 are chunk-slices of it (no separate h2 fp8 stream).
  - Biases: folded into matmuls via constant-1.0 rows in the zero padding
    (zr1@row657, nh1@row400-of-aligned-view, zr2/nh2@row912) or carried by
    ACT bias (tanh: bin; fc4: sigmoid) / DVE tensor_scalar (fc2/fc3 relu).
  - n-gate: psh=Whn@h+bhn accumulates in PSUM, DVE multiplies r in-place in
    the bank, then Wnx@x accumulates on top (start=False), ACT tanh reads the
    bank with bias=bin. Saves two scalar_tensor_tensor per chunk.
  - h' blend fused as three [128,4,nb] ops (sub/mul/add) writing fp8.
  - fc(t-1) matmul groups are interleaved into tile t's GRU n-chains to keep
    PE busy during the DVE/ACT latency chain (1-tile software pipeline).
"""

import sys

import numpy as np
import ml_dtypes

sys.path.insert(0, "/opt/trn_rl_repo")

import concourse.bacc as bacc
import concourse.mybir as mybir
import concourse.tile as tile
from concourse.bass import ts
from concourse.bass_utils import run_bass_kernel_spmd

BF16 = ml_dtypes.bfloat16
FP8 = ml_dtypes.float8_e4m3

B, F, H, FF = 32768, 257, 400, 600
NCORES = 8
BPC = B // NCORES           # 4096 batch rows per core
NB = 512                    # matmul free-dim tile (one PSUM bank of fp32)
XH1 = 769                   # [x(257) | h1(400) | 1-row(657) | 0] rows
ZRM = 1024                  # [z(400)|pad | r(400)|pad] aligned gate cols

AF = mybir.ActivationFunctionType
ALU = mybir.AluOpType
DR = mybir.MatmulPerfMode.DoubleRow

# packed fp32 bias columns: name -> (offset, n_chunks)
BIAS_LAYOUT = {}
_off = 0
for _n, _c in (("bnx1", 4), ("bnx2", 4), ("bfc2", 5), ("bfc3", 5), ("bfc4", 3)):
    BIAS_LAYOUT[_n] = (_off, _c)
    _off += _c
BIAS_COLS = _off


def _pad2(a, rows, cols):
    out = np.zeros((rows, cols), dtype=np.float64)
    out[: a.shape[0], : a.shape[1]] = a
    return out


def _bias_tile(vec, padded):
    v = np.zeros(padded, dtype=np.float64)
    v[: vec.shape[0]] = vec
    return np.ascontiguousarray(v.reshape(padded // 128, 128).T).astype(np.float32)


def prepare_weights(inp):
    f64 = {k: np.asarray(v, dtype=np.float64) for k, v in inp.items()}
    w = {}

    # fc1 fold for GRU1 input side
    Wx, bx = {}, {}
    for name in ("z", "r", "n"):
        Wx[name] = (f64[f"Wi{name}1"] @ f64["Wfc1"]).T          # [F, H]
        bx[name] = f64[f"bi{name}1"] + f64[f"Wi{name}1"] @ f64["bfc1"]

    # GRU1 z,r: K = [x(257) | h1(400) | 1@657 | x256@658] -> 768 rows
    # (6 chunks), M = [z | pad | r | pad] = 1024 (lane-aligned gates).
    Wzr1 = np.zeros((768, ZRM), dtype=np.float64)
    for g, name in enumerate(("z", "r")):
        c0 = 512 * g
        Wzr1[:F, c0 : c0 + H] = Wx[name]
        Wzr1[F : F + H, c0 : c0 + H] = f64[f"Wh{name}1"].T
        Wzr1[657, c0 : c0 + H] = bx[name] + f64[f"bh{name}1"]
    w["Wzr1"] = Wzr1
    # GRU1 n input side: rhs = xh chunks 0..1 (rows 0..255). x feature 256
    # rides in the nh1 operand instead (xh row 658 = x[:,256] on host).
    w["Wn1x"] = _pad2(Wx["n"][:256], 256, 512)
    # GRU1 n hidden side: rhs = aligned h1 view (xh rows 257..768):
    # pos 400 = 1-row (bhn1), pos 401 = x[:,256] (row 658).
    Wn1h = _pad2(f64["Whn1"].T, 512, 512)
    Wn1h[400, :H] = f64["bhn1"]
    Wn1h[401, :H] = Wx["n"][256]
    w["Wn1h"] = Wn1h

    # GRU2 z,r: K = [g1(512-pad) | h2(400) | 1@912] = 1024 (8 chunks)
    Wzr2 = np.zeros((1024, ZRM), dtype=np.float64)
    for g, name in enumerate(("z", "r")):
        c0 = 512 * g
        Wzr2[:H, c0 : c0 + H] = f64[f"Wi{name}2"].T
        Wzr2[512 : 512 + H, c0 : c0 + H] = f64[f"Wh{name}2"].T
        Wzr2[912, c0 : c0 + H] = f64[f"bi{name}2"] + f64[f"bh{name}2"]
    w["Wzr2"] = Wzr2
    w["Wn2x"] = _pad2(f64["Win2"].T, 512, 512)
    Wn2h = _pad2(f64["Whn2"].T, 512, 512)
    Wn2h[400, :H] = f64["bhn2"]
    w["Wn2h"] = Wn2h

    w["Wfc2"] = _pad2(f64["Wfc2"].T, 512, 640)
    w["Wfc3"] = _pad2(f64["Wfc3"].T, 768, 640)
    w["Wfc4"] = _pad2(f64["Wfc4"].T, 768, 384)

    weights = {k: np.ascontiguousarray(v).astype(FP8) for k, v in w.items()}

    parts = [
        ("bnx1", _bias_tile(bx["n"], 512)),
        ("bnx2", _bias_tile(f64["bin2"], 512)),
        ("bfc2", _bias_tile(f64["bfc2"], 640)),
        ("bfc3", _bias_tile(f64["bfc3"], 640)),
        ("bfc4", _bias_tile(f64["bfc4"], 384)),
    ]
    biases = {"biasT": np.concatenate([p[1] for p in parts], axis=1)}
    return weights, biases


def build_nc(nbt=BPC, nb=NB):
    nc = bacc.Bacc("TRN2", target_bir_lowering=False, debug=False)
    bf = mybir.dt.bfloat16
    f32 = mybir.dt.float32
    f8 = mybir.dt.float8e4

    xh8 = nc.declare_dram_parameter("xh8", [XH1, nbt], f8, isOutput=False)
    h28 = nc.declare_dram_parameter("h28", [512, nbt], f8, isOutput=False)
    h1T = nc.declare_dram_parameter("h1T", [512, nbt], bf, isOutput=False)
    h2T = nc.declare_dram_parameter("h2T", [512, nbt], bf, isOutput=False)
    wd = {}
    for name, k, m in (
        ("Wzr1", 768, ZRM), ("Wn1x", 256, 512), ("Wn1h", 512, 512),
        ("Wzr2", 1024, ZRM), ("Wn2x", 512, 512), ("Wn2h", 512, 512),
        ("Wfc2", 512, 640), ("Wfc3", 768, 640), ("Wfc4", 768, 384),
    ):
        wd[name] = nc.declare_dram_parameter(name, [k, m], f8, isOutput=False)
    biasT_d = nc.declare_dram_parameter("biasT", [128, BIAS_COLS], f32, isOutput=False)
    outT = nc.declare_dram_parameter("outT", [384, nbt], bf, isOutput=True)

    n_tiles = nbt // nb

    with tile.TileContext(nc) as tc:
        with (
            tc.tile_pool(name="wpool", bufs=1) as wpool,
            tc.tile_pool(name="bpool", bufs=1) as bpool,
            tc.tile_pool(name="io", bufs=3) as io,
            tc.tile_pool(name="inp", bufs=3) as inp,
            tc.tile_pool(name="act", bufs=2) as act,
            tc.tile_pool(name="tmp", bufs=2) as tmp,
            tc.tile_pool(name="pzr1", bufs=2, space="PSUM") as pzr1,
            tc.tile_pool(name="pzr2", bufs=2, space="PSUM") as pzr2,
            tc.tile_pool(name="pn", bufs=3, space="PSUM") as pn,
            tc.tile_pool(name="pfc", bufs=1, space="PSUM") as pfc,
        ):
            W, BT = {}, {}

            def poke(t, dep):
                """Write one corner element of t from dep: makes t's DMA
                (WAW) wait until dep's transfer has fully landed. This is a
                real data dependency, so the list scheduler cannot hoist the
                DMA ahead of it the way it can with a ring-order gate."""
                nc.gpsimd.tensor_scalar_add(t[0:1, 0, 0:1], dep[0:1, 0, 0:1],
                                            0.0)

            def load_w(name, eng=None, dep=None):
                dram = wd[name]
                k, m = dram.shape
                t = wpool.tile([128, k // 128, m], dram.dtype, tag=name)
                if dep is not None:
                    poke(t, dep)
                r = dram.rearrange("(c p) m -> p c m", p=128)
                (eng or nc.sync).dma_start(out=t, in_=r)
                W[name] = t

            def load_bias():
                biasT = bpool.tile([128, BIAS_COLS], f32, tag="biasT")
                nc.sync.dma_start(out=biasT, in_=biasT_d[:, :])
                for _n, (_o, _c) in BIAS_LAYOUT.items():
                    BT[_n] = biasT[:, _o : _o + _c]

            xh_zr = xh8[0:768, :].rearrange("(c p) n -> p c n", p=128)
            h1m_al = xh8[257 : 257 + 512, :].rearrange("(c p) n -> p c n", p=128)
            h28_al = h28.rearrange("(c p) n -> p c n", p=128)
            h1_bl = h1T.rearrange("(c p) n -> p c n", p=128)
            h2_bl = h2T.rearrange("(c p) n -> p c n", p=128)
            outT_r = outT.rearrange("(c p) n -> p c n", p=128)

            def load_inputs(t, crit_only=False, rest=None, dep=None):
                """crit_only: issue just xh+h1m (stage 1), return partial;
                rest: finish a partial load; dep: poke-gate every DMA."""
                sl = ts(t, nb)

                def ld(shape, dt_, tag, src):
                    tl = inp.tile(shape, dt_, tag=tag)
                    if dep is not None:
                        poke(tl, dep)
                    nc.sync.dma_start(out=tl, in_=src)
                    return tl

                if rest is None:
                    xh = ld([128, 6, nb], f8, "xh", xh_zr[:, :, sl])
                    h1m = ld([128, 4, nb], f8, "h1m", h1m_al[:, :, sl])
                    if crit_only:
                        return [xh, h1m]
                else:
                    xh, h1m = rest
                h1s = ld([128, 4, nb], bf, "h1s", h1_bl[:, :, sl])
                h2s = ld([128, 4, nb], bf, "h2s", h2_bl[:, :, sl])
                # GRU2 K operand [g1(512) | h2(512)] as two tiles so the h2
                # half's matmuls don't wait on GRU1's blend.
                zr2h = ld([128, 4, nb], f8, "zr2h", h28_al[:, :, sl])
                zr2g = io.tile([128, 4, nb], f8, tag="zr2g")
                return xh, h1m, h1s, h2s, zr2g, zr2h

            def matseq_dr(ps, Wt, col, segs, start0=True):
                """Accumulate into ps via DoubleRow passes. segs: list of
                (w_chunk0, rhs_tile, rhs_chunk0, n_pairs)."""
                total = sum(s[3] for s in segs)
                i = 0
                for wk0, rhs_t, rk0, npairs in segs:
                    for j in range(npairs):
                        wk, rk = wk0 + 2 * j, rk0 + 2 * j
                        nc.tensor.matmul(
                            ps, Wt[:, wk : wk + 2, col : col + 128],
                            rhs_t[:, rk : rk + 2, :],
                            start=(start0 and i == 0), stop=(i == total - 1),
                            perf_mode=DR,
                            skip_group_check=(not start0))
                        i += 1

            def zr_items(pool, tag, Wzr, segs, zro):
                """8 gate chunks as single-bank group closures, r chunks
                (4..7) first. Unpaired: finer pipeline + fewer banks held."""
                def item(c):
                    def run():
                        ps = pool.tile([128, nb], f32, tag=tag)
                        matseq_dr(ps, Wzr, c * 128, segs)
                        nc.scalar.activation(zro[:, c, :], ps, AF.Sigmoid)
                    return run
                return [item(c) for c in (4, 5, 6, 7, 0, 1, 2, 3)]

            def n_chain(Wnh, nh_segs, Wnx, nx_segs, zro, bnx, n_t, filler):
                """n = tanh(Wnx@x + bnx + r*(Wnh@h + bhn)); r = zro[:,4+c,:].
                Runs filler() work items between chunks to keep PE fed."""
                for c in range(4):
                    ps = pn.tile([128, nb], f32, tag="ps_n")
                    matseq_dr(ps, Wnh, c * 128, nh_segs)
                    nc.vector.tensor_mul(ps, ps, zro[:, 4 + c, :])
                    filler()
                    matseq_dr(ps, Wnx, c * 128, nx_segs, start0=False)
                    nc.scalar.activation(n_t[:, c, :], ps, AF.Tanh,
                                         bias=bnx[:, c : c + 1])
                    filler()

            def blend(n_t, h_s, zro, out_t, tag):
                # two chunk-pair halves -> consumers of half 0 unblock before
                # the second half's tanh has even finished
                for h in (0, 1):
                    sl = slice(2 * h, 2 * h + 2)
                    d = tmp.tile([128, 2, nb], bf, tag=f"d{tag}{h}")
                    nc.vector.tensor_sub(d, h_s[:, sl, :], n_t[:, sl, :])
                    zd = tmp.tile([128, 2, nb], bf, tag=f"zd{tag}{h}")
                    nc.vector.tensor_mul(zd, zro[:, sl, :], d)
                    nc.vector.tensor_add(out_t[:, sl, :], n_t[:, sl, :], zd)

            def fc_items(g2, f2, f3, o):
                """List of closures: one PSUM group + activation each."""
                items = []

                def fc_item(Wt, kc, rhs, m, kind, out_ap, bias):
                    def run():
                        ps = pfc.tile([128, nb], f32, tag="ps_fc")
                        matseq_dr(ps, Wt, m * 128, [(0, rhs, 0, kc // 2)])
                        if kind == "relu":
                            nc.vector.tensor_scalar(
                                out_ap, ps, bias, 0.0, op0=ALU.add, op1=ALU.max)
                        else:
                            nc.scalar.activation(out_ap, ps, AF.Sigmoid,
                                                 bias=bias)
                    return run

                for m in range(5):
                    items.append(fc_item(W["Wfc2"], 4, g2, m, "relu",
                                         f2[:, m, :], BT["bfc2"][:, m : m + 1]))
                for m in range(5):
                    items.append(fc_item(W["Wfc3"], 6, f2, m, "relu",
                                         f3[:, m, :], BT["bfc3"][:, m : m + 1]))
                for m in range(3):
                    items.append(fc_item(W["Wfc4"], 6, f3, m, "sig",
                                         o[:, m, :], BT["bfc4"][:, m : m + 1]))
                return items

            # Startup staging: the DMA hw round-robins ALL outstanding
            # transfers, so the first-needed data must be the ONLY data in
            # flight. Stage 1 (ungated): everything GRU1(0) needs. Later
            # stages are held back by tiny gate reads on their issue rings —
            # an in-order ring can't issue its next DMA until the gate's
            # input tile has fully landed.
            # Stage 1 — ONLY what GRU1(0) needs up to the n-chain: xh, h1m,
            # Wzr1, Wn1h, Wn1x (~1.7MB -> all landed by ~5us).
            load_w("Wzr1", eng=nc.scalar)
            crit0 = load_inputs(0, crit_only=True)        # sync: xh, h1m
            load_w("Wn1h", eng=nc.gpsimd)
            load_w("Wn1x", eng=nc.gpsimd)
            # ACT-table warmup (sigmoid+tanh) before the first gate sigmoid
            warm = bpool.tile([128, 1], f32, tag="warm")
            nc.vector.memset(warm, 0.0)
            nc.scalar.activation(warm, warm, AF.Sigmoid)
            nc.scalar.activation(warm, warm, AF.Tanh)

            # stage 2 (poke-gated on stage-1 tiles): Wzr2, rest of tile0's
            # inputs, n2 weights. bias (128 tiny descriptors, ~no bandwidth)
            # trickles ungated.
            load_bias()
            load_w("Wzr2", eng=nc.scalar, dep=crit0[0])
            ins0 = load_inputs(0, rest=crit0, dep=crit0[1])
            for name in ("Wn2x", "Wn2h"):
                load_w(name, eng=nc.gpsimd, dep=crit0[1])
            # stage 3 (gated on stage 2): fc weights, needed ~35us in
            for name in ("Wfc2", "Wfc3", "Wfc4"):
                load_w(name, eng=nc.gpsimd, dep=W["Wzr2"])

            fcq = []            # fc work items from tile t-1
            zr1q = []           # prefetched zr1(t) pair items
            prev_out = None     # (o_tile, t-1) awaiting DMA out
            tiles_in = {0: ins0}

            for t in range(n_tiles):
                xh, h1m, h1s, h2s, zr2g, zr2h = tiles_in.pop(t)
                if t + 1 < n_tiles:     # prefetch next tile's inputs now;
                    # tile1's prefetch is poke-gated behind the startup
                    tiles_in[t + 1] = load_inputs(
                        t + 1, dep=W["Wzr2"] if t == 0 else None)

                def fc_fill(k=2):
                    for _ in range(k):
                        if fcq:
                            fcq.pop(0)()

                # ---- GRU1 ----
                if t == 0:
                    zro1 = act.tile([128, 8, nb], bf, tag="zro1")
                    for it in zr_items(pzr1, "ps_zr1", W["Wzr1"],
                                       [(0, xh, 0, 3)], zro1):
                        it()
                else:
                    zro1 = zro1_next  # noqa: F821  (emitted last iteration)
                n1 = act.tile([128, 4, nb], bf, tag="n1")
                n_chain(W["Wn1h"], [(0, h1m, 0, 2)], W["Wn1x"],
                        [(0, xh, 0, 1)], zro1, BT["bnx1"], n1, fc_fill)
                blend(n1, h1s, zro1, zr2g, "1")

                # ---- GRU2 zr r-chunks (critical: feed n-chain2), then
                # zr1(t+1) on its own PSUM pool — the scheduler runs those
                # fully-ready groups whenever blend1 blocks zr2's g1 passes
                zro2 = act.tile([128, 8, nb], bf, tag="zro2")
                zr2_segs = [(4, zr2h, 0, 2), (0, zr2g, 0, 2)]
                zr2_all = zr_items(pzr2, "ps_zr2", W["Wzr2"], zr2_segs, zro2)
                for it in zr2_all[:4]:      # r chunks 4..7
                    it()
                    fc_fill(1)
                if t + 1 < n_tiles:
                    zro1_next = act.tile([128, 8, nb], bf, tag="zro1")
                    for it in zr_items(pzr1, "ps_zr1", W["Wzr1"],
                                       [(0, tiles_in[t + 1][0], 0, 3)],
                                       zro1_next):
                        it()
                        fc_fill(1)

                zq = zr2_all[4:] + fcq      # z chunks + leftover fc(t-1)
                fcq = []

                def z_fill():
                    if zq:
                        zq.pop(0)()

                # ---- GRU2 n-chain ----
                g2 = io.tile([128, 4, nb], f8, tag="g2")
                n2 = act.tile([128, 4, nb], bf, tag="n2")
                n_chain(W["Wn2h"], [(0, zr2h, 0, 2)], W["Wn2x"],
                        [(0, zr2g, 0, 2)], zro2, BT["bnx2"], n2, z_fill)
                while zq:
                    zq.pop(0)()
                blend(n2, h2s, zro2, g2, "2")

                if prev_out is not None:
                    o_prev, tp = prev_out
                    nc.sync.dma_start(out=outT_r[:, :, ts(tp, nb)], in_=o_prev)

                # ---- queue this tile's fc stage ----
                f2 = io.tile([128, 6, nb], f8, tag="f2")
                f3 = io.tile([128, 6, nb], f8, tag="f3")
                if t < 3:   # io pool bufs=3: zero the K-pad chunk once per buf
                    nc.gpsimd.memset(f2[:, 5, :], 0.0)
                    nc.gpsimd.memset(f3[:, 5, :], 0.0)
                o = io.tile([128, 3, nb], bf, tag="o")
                fcq = fc_items(g2, f2, f3, o)
                prev_out = (o, t)

            while fcq:
                fcq.pop(0)()
            o_last, tl = prev_out
            nc.sync.dma_start(out=outT_r[:, :, ts(tl, nb)], in_=o_last)

    nc.compile()
    return nc


def _shard_inputs(inp, weights, biases):
    x = np.asarray(inp["x"], dtype=np.float32)
    h1 = np.asarray(inp["h1"], dtype=np.float32)
    h2 = np.asarray(inp["h2"], dtype=np.float32)

    xh8 = np.zeros((NCORES, XH1, BPC), dtype=FP8)
    h28 = np.zeros((NCORES, 512, BPC), dtype=FP8)
    h1T = np.zeros((NCORES, 512, BPC), dtype=BF16)
    h2T = np.zeros((NCORES, 512, BPC), dtype=BF16)
    for i in range(NCORES):
        sl = slice(i * BPC, (i + 1) * BPC)
        xh8[i, :F] = x[sl].T.astype(FP8)
        xh8[i, F : F + H] = h1[sl].T.astype(FP8)
        xh8[i, 657] = 1.0
        xh8[i, 658] = x[sl, 256].astype(FP8)  # x feat 256 rides in nh1's K
        h28[i, :H] = h2[sl].T.astype(FP8)
        h28[i, 400] = 1.0
        h1T[i, :H] = h1[sl].T.astype(BF16)
        h2T[i, :H] = h2[sl].T.astype(BF16)

    in_maps = []
    for i in range(NCORES):
        m = {"xh8": xh8[i], "h28": h28[i], "h1T": h1T[i], "h2T": h2T[i]}
        m.update(weights)
        m.update(biases)
        in_maps.append(m)
    return in_maps


def _run(inp, trace=False):
    weights, biases = prepare_weights(inp)
    nc = build_nc()
    in_maps = _shard_inputs(inp, weights, biases)
    res = run_bass_kernel_spmd(nc, in_maps, list(range(NCORES)), trace=trace)
    out = np.empty((B, F), dtype=np.float32)
    for i in range(NCORES):
        out[i * BPC : (i + 1) * BPC] = (
            np.asarray(res.results[i]["outT"][:F]).astype(np.float32).T
        )
    return out, res


def kernel(**inputs) -> np.ndarray:
    out, _ = _run(inputs, trace=False)
    return out


# revision 39
# speedup vs baseline: 1.2530x; 1.0084x over previous
"""NsNet2 single-step (fc1 + 2x GRU cell + 3x FC) Trainium2 kernel, v2.

Strategy (v2 — all-fp8 DoubleRow):
  - Pure data parallel: B=32768 -> 4096 rows/core on 8 cores; feature-major
    on chip ([feat, batch]) so matmul operands need no on-chip transposes.
  - EVERY matmul is fp8e4m3 + DoubleRow (0.5 cyc/row) with even K-chunk
    counts (K pads to 2-chunk multiples); numpy-sim predicts rel_err 1.6e-2.
  - Gate layout: zr weights have M=1024 = [z(400)|pad|r(400)|pad] so both
    gates land lane-aligned -> no r-realign DMAs. r-pairs are computed first
    so the n-gate chain can start early.
  - GRU2 contraction operand is one aligned block zr2op = [g1(512)|h2(512)]:
    chunks 0..3 written by GRU1's blend (fp8), chunks 4..7 DMA'd from padded
    h2; nh2/nx2 are chunk-slices of it (no separate h2 fp8 stream).
  - Biases: folded into matmuls via constant-1.0 rows in the zero padding
    (zr1@row657, nh1@row400-of-aligned-view, zr2/nh2@row912) or carried by
    ACT bias (tanh: bin; fc4: sigmoid) / DVE tensor_scalar (fc2/fc3 relu).
  - n-gate: psh=Whn@h+bhn accumulates in PSUM, DVE multiplies r in-place in
    the bank, then Wnx@x accumulates on top (start=False), ACT tanh reads the
    bank with bias=bin. Saves two scalar_tensor_tensor per chunk.
  - h' blend fused as three [128,4,nb] ops (sub/mul/add) writing fp8.
  - fc(t-1) matmul groups are interleaved into tile t's GRU n-chains to keep
    PE busy during the DVE/ACT latency chain (1-tile software pipeline).
"""

import sys

import numpy as np
import ml_dtypes

sys.path.insert(0, "/opt/trn_rl_repo")

import concourse.bacc as bacc
import concourse.mybir as mybir
import concourse.tile as tile
from concourse.bass import ts
from concourse.bass_utils import run_bass_kernel_spmd

BF16 = ml_dtypes.bfloat16
FP8 = ml_dtypes.float8_e4m3

B, F, H, FF = 32768, 257, 400, 600
NCORES = 8
BPC = B // NCORES           # 4096 batch rows per core
NB = 512                    # matmul free-dim tile (one PSUM bank of fp32)
XH1 = 769                   # [x(257) | h1(400) | 1-row(657) | 0] rows
ZRM = 1024                  # [z(400)|pad | r(400)|pad] aligned gate cols

AF = mybir.ActivationFunctionType
ALU = mybir.AluOpType
DR = mybir.MatmulPerfMode.DoubleRow

# packed fp32 bias columns: name -> (offset, n_chunks)
BIAS_LAYOUT = {}
_off = 0
for _n, _c in (("bnx1", 4), ("bnx2", 4), ("bfc2", 5), ("bfc3", 5), ("bfc4", 3)):
    BIAS_LAYOUT[_n] = (_off, _c)
    _off += _c
BIAS_COLS = _off


def _pad2(a, rows, cols):
    out = np.zeros((rows, cols), dtype=np.float64)
    out[: a.shape[0], : a.shape[1]] = a
    return out


def _bias_tile(vec, padded):
    v = np.zeros(padded, dtype=np.float64)
    v[: vec.shape[0]] = vec
    return np.ascontiguousarray(v.reshape(padded // 128, 128).T).astype(np.float32)


def prepare_weights(inp):
    f64 = {k: np.asarray(v, dtype=np.float64) for k, v in inp.items()}
    w = {}

    # fc1 fold for GRU1 input side
    Wx, bx = {}, {}
    for name in ("z", "r", "n"):
        Wx[name] = (f64[f"Wi{name}1"] @ f64["Wfc1"]).T          # [F, H]
        bx[name] = f64[f"bi{name}1"] + f64[f"Wi{name}1"] @ f64["bfc1"]

    # GRU1 z,r: K = [x(257) | h1(400) | 1@657 | x256@658] -> 768 rows
    # (6 chunks), M = [z | pad | r | pad] = 1024 (lane-aligned gates).
    Wzr1 = np.zeros((768, ZRM), dtype=np.float64)
    for g, name in enumerate(("z", "r")):
        c0 = 512 * g
        Wzr1[:F, c0 : c0 + H] = Wx[name]
        Wzr1[F : F + H, c0 : c0 + H] = f64[f"Wh{name}1"].T
        Wzr1[657, c0 : c0 + H] = bx[name] + f64[f"bh{name}1"]
    w["Wzr1"] = Wzr1
    # GRU1 n input side: rhs = xh chunks 0..1 (rows 0..255). x feature 256
    # rides in the nh1 operand instead (xh row 658 = x[:,256] on host).
    w["Wn1x"] = _pad2(Wx["n"][:256], 256, 512)
    # GRU1 n hidden side: rhs = aligned h1 view (xh rows 257..768):
    # pos 400 = 1-row (bhn1), pos 401 = x[:,256] (row 658).
    Wn1h = _pad2(f64["Whn1"].T, 512, 512)
    Wn1h[400, :H] = f64["bhn1"]
    Wn1h[401, :H] = Wx["n"][256]
    w["Wn1h"] = Wn1h

    # GRU2 z,r: K = [g1(512-pad) | h2(400) | 1@912] = 1024 (8 chunks)
    Wzr2 = np.zeros((1024, ZRM), dtype=np.float64)
    for g, name in enumerate(("z", "r")):
        c0 = 512 * g
        Wzr2[:H, c0 : c0 + H] = f64[f"Wi{name}2"].T
        Wzr2[512 : 512 + H, c0 : c0 + H] = f64[f"Wh{name}2"].T
        Wzr2[912, c0 : c0 + H] = f64[f"bi{name}2"] + f64[f"bh{name}2"]
    w["Wzr2"] = Wzr2
    w["Wn2x"] = _pad2(f64["Win2"].T, 512, 512)
    Wn2h = _pad2(f64["Whn2"].T, 512, 512)
    Wn2h[400, :H] = f64["bhn2"]
    w["Wn2h"] = Wn2h

    w["Wfc2"] = _pad2(f64["Wfc2"].T, 512, 640)
    w["Wfc3"] = _pad2(f64["Wfc3"].T, 768, 640)
    w["Wfc4"] = _pad2(f64["Wfc4"].T, 768, 384)

    weights = {k: np.ascontiguousarray(v).astype(FP8) for k, v in w.items()}

    parts = [
        ("bnx1", _bias_tile(bx["n"], 512)),
        ("bnx2", _bias_tile(f64["bin2"], 512)),
        ("bfc2", _bias_tile(f64["bfc2"], 640)),
        ("bfc3", _bias_tile(f64["bfc3"], 640)),
        ("bfc4", _bias_tile(f64["bfc4"], 384)),
    ]
    biases = {"biasT": np.concatenate([p[1] for p in parts], axis=1)}
    return weights, biases


def build_nc(nbt=BPC, nb=NB):
    nc = bacc.Bacc("TRN2", target_bir_lowering=False, debug=False)
    bf = mybir.dt.bfloat16
    f32 = mybir.dt.float32
    f8 = mybir.dt.float8e4

    xh8 = nc.declare_dram_parameter("xh8", [XH1, nbt], f8, isOutput=False)
    h28 = nc.declare_dram_parameter("h28", [512, nbt], f8, isOutput=False)
    h1T = nc.declare_dram_parameter("h1T", [512, nbt], bf, isOutput=False)
    h2T = nc.declare_dram_parameter("h2T", [512, nbt], bf, isOutput=False)
    wd = {}
    for name, k, m in (
        ("Wzr1", 768, ZRM), ("Wn1x", 256, 512), ("Wn1h", 512, 512),
        ("Wzr2", 1024, ZRM), ("Wn2x", 512, 512), ("Wn2h", 512, 512),
        ("Wfc2", 512, 640), ("Wfc3", 768, 640), ("Wfc4", 768, 384),
    ):
        wd[name] = nc.declare_dram_parameter(name, [k, m], f8, isOutput=False)
    biasT_d = nc.declare_dram_parameter("biasT", [128, BIAS_COLS], f32, isOutput=False)
    outT = nc.declare_dram_parameter("outT", [384, nbt], bf, isOutput=True)

    n_tiles = nbt // nb

    with tile.TileContext(nc) as tc:
        with (
            tc.tile_pool(name="wpool", bufs=1) as wpool,
            tc.tile_pool(name="bpool", bufs=1) as bpool,
            tc.tile_pool(name="io", bufs=3) as io,
            tc.tile_pool(name="inp", bufs=3) as inp,
            tc.tile_pool(name="act", bufs=2) as act,
            tc.tile_pool(name="tmp", bufs=2) as tmp,
            tc.tile_pool(name="pzr1", bufs=2, space="PSUM") as pzr1,
            tc.tile_pool(name="pzr2", bufs=2, space="PSUM") as pzr2,
            tc.tile_pool(name="pn", bufs=2, space="PSUM") as pn,
            tc.tile_pool(name="pfc", bufs=2, space="PSUM") as pfc,
        ):
            W, BT = {}, {}

            def poke(t, dep):
                """Write one corner element of t from dep: makes t's DMA
                (WAW) wait until dep's transfer has fully landed. This is a
                real data dependency, so the list scheduler cannot hoist the
                DMA ahead of it the way it can with a ring-order gate."""
                nc.gpsimd.tensor_scalar_add(t[0:1, 0, 0:1], dep[0:1, 0, 0:1],
                                            0.0)

            def load_w(name, eng=None, dep=None):
                dram = wd[name]
                k, m = dram.shape
                t = wpool.tile([128, k // 128, m], dram.dtype, tag=name)
                if dep is not None:
                    poke(t, dep)
                r = dram.rearrange("(c p) m -> p c m", p=128)
                (eng or nc.sync).dma_start(out=t, in_=r)
                W[name] = t

            def load_bias(dep=None):
                biasT = bpool.tile([128, BIAS_COLS], f32, tag="biasT")
                if dep is not None:
                    # 128 tiny descriptors would round-robin 1:1 against the
                    # critical startup streams; hold them behind stage 1
                    nc.gpsimd.tensor_scalar_add(biasT[0:1, 0:1],
                                                dep[0:1, 0, 0:1], 0.0)
                nc.sync.dma_start(out=biasT, in_=biasT_d[:, :])
                for _n, (_o, _c) in BIAS_LAYOUT.items():
                    BT[_n] = biasT[:, _o : _o + _c]

            xh_zr = xh8[0:768, :].rearrange("(c p) n -> p c n", p=128)
            h1m_al = xh8[257 : 257 + 512, :].rearrange("(c p) n -> p c n", p=128)
            h28_al = h28.rearrange("(c p) n -> p c n", p=128)
            h1_bl = h1T.rearrange("(c p) n -> p c n", p=128)
            h2_bl = h2T.rearrange("(c p) n -> p c n", p=128)
            outT_r = outT.rearrange("(c p) n -> p c n", p=128)

            def load_inputs(t, crit_only=False, rest=None, dep=None):
                """crit_only: issue just xh+h1m (stage 1), return partial;
                rest: finish a partial load; dep: poke-gate every DMA."""
                sl = ts(t, nb)

                def ld(shape, dt_, tag, src):
                    tl = inp.tile(shape, dt_, tag=tag)
                    if dep is not None:
                        poke(tl, dep)
                    nc.sync.dma_start(out=tl, in_=src)
                    return tl

                if rest is None:
                    xh = ld([128, 6, nb], f8, "xh", xh_zr[:, :, sl])
                    h1m = ld([128, 4, nb], f8, "h1m", h1m_al[:, :, sl])
                    if crit_only:
                        return [xh, h1m]
                else:
                    xh, h1m = rest
                h1s = ld([128, 4, nb], bf, "h1s", h1_bl[:, :, sl])
                h2s = ld([128, 4, nb], bf, "h2s", h2_bl[:, :, sl])
                # GRU2 K operand [g1(512) | h2(512)] as two tiles so the h2
                # half's matmuls don't wait on GRU1's blend.
                zr2h = ld([128, 4, nb], f8, "zr2h", h28_al[:, :, sl])
                zr2g = io.tile([128, 4, nb], f8, tag="zr2g")
                return xh, h1m, h1s, h2s, zr2g, zr2h

            def matseq_dr(ps, Wt, col, segs, start0=True):
                """Accumulate into ps via DoubleRow passes. segs: list of
                (w_chunk0, rhs_tile, rhs_chunk0, n_pairs)."""
                total = sum(s[3] for s in segs)
                i = 0
                for wk0, rhs_t, rk0, npairs in segs:
                    for j in range(npairs):
                        wk, rk = wk0 + 2 * j, rk0 + 2 * j
                        nc.tensor.matmul(
                            ps, Wt[:, wk : wk + 2, col : col + 128],
                            rhs_t[:, rk : rk + 2, :],
                            start=(start0 and i == 0), stop=(i == total - 1),
                            perf_mode=DR,
                            skip_group_check=(not start0))
                        i += 1

            def zr_items(pool, tag, Wzr, segs, zro):
                """8 gate chunks as single-bank group closures, r chunks
                (4..7) first. Unpaired: finer pipeline + fewer banks held."""
                def item(c):
                    def run():
                        ps = pool.tile([128, nb], f32, tag=tag)
                        matseq_dr(ps, Wzr, c * 128, segs)
                        nc.scalar.activation(zro[:, c, :], ps, AF.Sigmoid)
                    return run
                return [item(c) for c in (4, 5, 6, 7, 0, 1, 2, 3)]

            def n_chain(Wnh, nh_segs, Wnx, nx_segs, zro, bnx, n_t, filler):
                """n = tanh(Wnx@x + bnx + r*(Wnh@h + bhn)); r = zro[:,4+c,:].
                Runs filler() work items between chunks to keep PE fed."""
                for c in range(4):
                    ps = pn.tile([128, nb], f32, tag="ps_n")
                    matseq_dr(ps, Wnh, c * 128, nh_segs)
                    nc.vector.tensor_mul(ps, ps, zro[:, 4 + c, :])
                    filler()
                    matseq_dr(ps, Wnx, c * 128, nx_segs, start0=False)
                    nc.scalar.activation(n_t[:, c, :], ps, AF.Tanh,
                                         bias=bnx[:, c : c + 1])
                    filler()

            def blend(n_t, h_s, zro, out_t, tag):
                # two chunk-pair halves -> consumers of half 0 unblock before
                # the second half's tanh has even finished
                for h in (0, 1):
                    sl = slice(2 * h, 2 * h + 2)
                    d = tmp.tile([128, 2, nb], bf, tag=f"d{tag}{h}")
                    nc.vector.tensor_sub(d, h_s[:, sl, :], n_t[:, sl, :])
                    zd = tmp.tile([128, 2, nb], bf, tag=f"zd{tag}{h}")
                    nc.vector.tensor_mul(zd, zro[:, sl, :], d)
                    nc.vector.tensor_add(out_t[:, sl, :], n_t[:, sl, :], zd)

            def fc_items(g2, f2, f3, o):
                """List of closures: one PSUM group + activation each."""
                items = []

                def fc_item(Wt, kc, rhs, m, kind, out_ap, bias):
                    def run():
                        ps = pfc.tile([128, nb], f32, tag="ps_fc")
                        matseq_dr(ps, Wt, m * 128, [(0, rhs, 0, kc // 2)])
                        if kind == "relu":
                            nc.vector.tensor_scalar(
                                out_ap, ps, bias, 0.0, op0=ALU.add, op1=ALU.max)
                        else:
                            nc.scalar.activation(out_ap, ps, AF.Sigmoid,
                                                 bias=bias)
                    return run

                for m in range(5):
                    items.append(fc_item(W["Wfc2"], 4, g2, m, "relu",
                                         f2[:, m, :], BT["bfc2"][:, m : m + 1]))
                for m in range(5):
                    items.append(fc_item(W["Wfc3"], 6, f2, m, "relu",
                                         f3[:, m, :], BT["bfc3"][:, m : m + 1]))
                for m in range(3):
                    items.append(fc_item(W["Wfc4"], 6, f3, m, "sig",
                                         o[:, m, :], BT["bfc4"][:, m : m + 1]))
                return items

            # Startup staging: the DMA hw round-robins ALL outstanding
            # transfers, so the first-needed data must be the ONLY data in
            # flight. Stage 1 (ungated): everything GRU1(0) needs. Later
            # stages are held back by tiny gate reads on their issue rings —
            # an in-order ring can't issue its next DMA until the gate's
            # input tile has fully landed.
            # Stage 1 — ONLY what GRU1(0) needs up to the n-chain: xh, h1m,
            # Wzr1, Wn1h, Wn1x (~1.7MB -> all landed by ~5us).
            load_w("Wzr1", eng=nc.scalar)
            crit0 = load_inputs(0, crit_only=True)        # sync: xh, h1m
            load_w("Wn1h", eng=nc.gpsimd)
            load_w("Wn1x", eng=nc.gpsimd)
            # ACT-table warmup (sigmoid+tanh) before the first gate sigmoid
            warm = bpool.tile([128, 1], f32, tag="warm")
            nc.vector.memset(warm, 0.0)
            nc.scalar.activation(warm, warm, AF.Sigmoid)
            nc.scalar.activation(warm, warm, AF.Tanh)

            # stage 2 (poke-gated on stage-1 tiles): Wzr2, rest of tile0's
            # inputs, n2 weights, bias
            load_bias(dep=crit0[1])
            load_w("Wzr2", eng=nc.scalar, dep=crit0[0])
            ins0 = load_inputs(0, rest=crit0, dep=crit0[1])
            for name in ("Wn2x", "Wn2h"):
                load_w(name, eng=nc.gpsimd, dep=crit0[1])
            # stage 3 (gated on stage 2): fc weights, needed ~35us in
            for name in ("Wfc2", "Wfc3", "Wfc4"):
                load_w(name, eng=nc.gpsimd, dep=W["Wzr2"])

            fcq = []            # fc work items from tile t-1
            zr1q = []           # prefetched zr1(t) pair items
            prev_out = None     # (o_tile, t-1) awaiting DMA out
            tiles_in = {0: ins0}

            for t in range(n_tiles):
                xh, h1m, h1s, h2s, zr2g, zr2h = tiles_in.pop(t)
                if t + 1 < n_tiles:     # prefetch next tile's inputs now;
                    # tile1's prefetch is poke-gated behind the startup
                    tiles_in[t + 1] = load_inputs(
                        t + 1, dep=W["Wzr2"] if t == 0 else None)

                def fc_fill(k=2):
                    for _ in range(k):
                        if fcq:
                            fcq.pop(0)()

                # ---- GRU1 ----
                if t == 0:
                    zro1 = act.tile([128, 8, nb], bf, tag="zro1")
                    for it in zr_items(pzr1, "ps_zr1", W["Wzr1"],
                                       [(0, xh, 0, 3)], zro1):
                        it()
                else:
                    zro1 = zro1_next  # noqa: F821  (emitted last iteration)
                n1 = act.tile([128, 4, nb], bf, tag="n1")
                n_chain(W["Wn1h"], [(0, h1m, 0, 2)], W["Wn1x"],
                        [(0, xh, 0, 1)], zro1, BT["bnx1"], n1, fc_fill)
                blend(n1, h1s, zro1, zr2g, "1")

                # ---- GRU2 zr r-chunks (critical: feed n-chain2), then
                # zr1(t+1) on its own PSUM pool — the scheduler runs those
                # fully-ready groups whenever blend1 blocks zr2's g1 passes
                zro2 = act.tile([128, 8, nb], bf, tag="zro2")
                zr2_segs = [(4, zr2h, 0, 2), (0, zr2g, 0, 2)]
                zr2_all = zr_items(pzr2, "ps_zr2", W["Wzr2"], zr2_segs, zro2)
                for it in zr2_all[:4]:      # r chunks 4..7
                    it()
                    fc_fill(1)
                if t + 1 < n_tiles:
                    zro1_next = act.tile([128, 8, nb], bf, tag="zro1")
                    for it in zr_items(pzr1, "ps_zr1", W["Wzr1"],
                                       [(0, tiles_in[t + 1][0], 0, 3)],
                                       zro1_next):
                        it()
                        fc_fill(1)

                zq = zr2_all[4:] + fcq      # z chunks + leftover fc(t-1)
                fcq = []

                def z_fill():
                    if zq:
                        zq.pop(0)()

                # ---- GRU2 n-chain ----
                g2 = io.tile([128, 4, nb], f8, tag="g2")
                n2 = act.tile([128, 4, nb], bf, tag="n2")
                n_chain(W["Wn2h"], [(0, zr2h, 0, 2)], W["Wn2x"],
                        [(0, zr2g, 0, 2)], zro2, BT["bnx2"], n2, z_fill)
                while zq:
                    zq.pop(0)()
                blend(n2, h2s, zro2, g2, "2")

                if prev_out is not None:
                    o_prev, tp = prev_out
                    nc.sync.dma_start(out=outT_r[:, :, ts(tp, nb)], in_=o_prev)

                # ---- queue this tile's fc stage ----
                f2 = io.tile([128, 6, nb], f8, tag="f2")
                f3 = io.tile([128, 6, nb], f8, tag="f3")
                if t < 3:   # io pool bufs=3: zero the K-pad chunk once per buf
                    nc.gpsimd.memset(f2[:, 5, :], 0.0)
                    nc.gpsimd.memset(f3[:, 5, :], 0.0)
                o = io.tile([128, 3, nb], bf, tag="o")
                fcq = fc_items(g2, f2, f3, o)
                prev_out = (o, t)

            while fcq:
                fcq.pop(0)()
            o_last, tl = prev_out
            nc.sync.dma_start(out=outT_r[:, :, ts(tl, nb)], in_=o_last)

    nc.compile()
    return nc


def _shard_inputs(inp, weights, biases):
    x = np.asarray(inp["x"], dtype=np.float32)
    h1 = np.asarray(inp["h1"], dtype=np.float32)
    h2 = np.asarray(inp["h2"], dtype=np.float32)

    xh8 = np.zeros((NCORES, XH1, BPC), dtype=FP8)
    h28 = np.zeros((NCORES, 512, BPC), dtype=FP8)
    h1T = np.zeros((NCORES, 512, BPC), dtype=BF16)
    h2T = np.zeros((NCORES, 512, BPC), dtype=BF16)
    for i in range(NCORES):
        sl = slice(i * BPC, (i + 1) * BPC)
        xh8[i, :F] = x[sl].T.astype(FP8)
        xh8[i, F : F + H] = h1[sl].T.astype(FP8)
        xh8[i, 657] = 1.0
        xh8[i, 658] = x[sl, 256].astype(FP8)  # x feat 256 rides in nh1's K
        h28[i, :H] = h2[sl].T.astype(FP8)
        h28[i, 400] = 1.0
        h1T[i, :H] = h1[sl].T.astype(BF16)
        h2T[i, :H] = h2[sl].T.astype(BF16)

    in_maps = []
    for i in range(NCORES):
        m = {"xh8": xh8[i], "h28": h28[i], "h1T": h1T[i], "h2T": h2T[i]}
        m.update(weights)
        m.update(biases)
        in_maps.append(m)
    return in_maps


def _run(inp, trace=False):
    weights, biases = prepare_weights(inp)
    nc = build_nc()
    in_maps = _shard_inputs(inp, weights, biases)
    res = run_bass_kernel_spmd(nc, in_maps, list(range(NCORES)), trace=trace)
    out = np.empty((B, F), dtype=np.float32)
    for i in range(NCORES):
        out[i * BPC : (i + 1) * BPC] = (
            np.asarray(res.results[i]["outT"][:F]).astype(np.float32).T
        )
    return out, res


def kernel(**inputs) -> np.ndarray:
    out, _ = _run(inputs, trace=False)
    return out
